# revision 19
# baseline (speedup 1.0000x reference)
"""Causal multi-head attention on 8 TRN2 NeuronCores.

Sharding: tensor-parallel over heads (2 heads/core) for QKV projection and
attention; AllToAll redistributes attention outputs so each core owns L/8
rows of every batch for the output projection. Full inputs in, full output
out; all FLOPs on device.

v2 changes vs the PE-transpose baseline (456us):
  * X^T and V' are produced by the DMA xbar transpose engine
    (dma_start_transpose, HWDGE/sync queue) instead of PE transpose-mode
    matmuls. This removes 576 transposes + their LDWEIGHTS (~200us of PE
    time at the throttled clock) and stops transpose-mode activity from
    suppressing the HAM activity credit (transpose-mode never counts as
    PE-busy, so interleaving it everywhere pinned the PE clock at 1.2GHz).
  * ScalarE runs ONLY the softmax exp activations; PSUM evictions moved to
    DVE (tensor_copy), QKV bias-adds stay on DVE.
  * Softmax denominator reciprocal via reciprocal_approx_fast (~5x faster
    than InstReciprocal which measured 12.9us per [64, 2048] tile).
  * V' built per head by one dense SBUF->SBUF xbar block-transpose from a
    padded staging tile that carries a ones row, so [V_h | ones] is a
    contiguous PV stationary and the softmax denominator falls out of the
    PV matmul for free.

Per-core pipeline (per batch):
  1. X^T tiles via xbar transpose DMA straight from DRAM; QKV projection
     with stationary W tiles -> QKVt [384c, L], bias added on DVE.
  2. V' block-transpose (one xbar op) + ones columns.
  3. Causal attention per head: S^T = K-stationary.T @ Q into fp32 PSUM,
     exp fused on ScalarE (scale=1/sqrt(d), no max subtraction -- scores
     are bounded for this problem), diagonal tiles masked on DVE,
     O'.T accumulated over key tiles in fp32 PSUM with an appended ones
     row; normalization deferred: numerators evicted unnormalized (DVE),
     denominator reciprocal broadcast via DRAM bounce, one DVE multiply.
  4. AllToAll (per batch): own-128-channel x 256-row blocks redistributed
     so each core gets all 1024 channels for its own L/8 rows.
  5. Output projection (stationary gathered O.T tiles, moving W_out) plus
     broadcast fp32 bias, DMA to the per-core output row slice.
  The output projection of batch b-1 is emitted after batch b's AllToAll
  launch; stage 1+2 of batch b+1 interleaves into batch b's attention.
"""
from contextlib import ExitStack

import ml_dtypes
import numpy as np

import concourse.bass as bass
import concourse.tile as tile
from concourse import bacc, mybir
from concourse.bass_utils import run_bass_kernel_spmd

N_CORES = 8
D_MODEL = 1024
N_HEADS = 16
D_ATTN = 64
SEQ_LEN = 2048
BATCH = 4

F32 = mybir.dt.float32
BF16 = mybir.dt.bfloat16
ActF = mybir.ActivationFunctionType
BF16_NP = np.dtype(ml_dtypes.bfloat16)


def build_program(B=BATCH, L=SEQ_LEN):
    """Build the SPMD Bass program. Parametric over batch/seq for sim tests.
    Requires L % 1024 == 0. Full problem: B=4, L=2048."""
    assert L % 1024 == 0
    D = D_MODEL
    CQ = 1024                 # l_q chunk for attention
    NCH = L // CQ             # number of l_q chunks (2 at L=2048)
    NJT = L // 128            # number of l_k tiles (16)
    LO = L // N_CORES         # own rows per batch per core (256)
    KT = D // 128             # contraction k-tiles (8)
    NDT = CQ // 128           # l_k tiles per diagonal chunk (8)

    nc = bacc.Bacc("TRN2", target_bir_lowering=False, debug=False,
                   num_devices=N_CORES)

    X = nc.dram_tensor("X", [B, L, D], BF16, kind="ExternalInput").ap()
    WQKV = nc.dram_tensor("WQKV", [D, 384], BF16, kind="ExternalInput").ap()
    BQKV = nc.dram_tensor("BQKV", [3, 128, 1], F32, kind="ExternalInput").ap()
    WOUT = nc.dram_tensor("WOUT", [D, D], BF16, kind="ExternalInput").ap()
    BOUT = nc.dram_tensor("BOUT", [D], F32, kind="ExternalInput").ap()
    MASKS = nc.dram_tensor("MASKS", [128, 128], BF16,
                           kind="ExternalInput").ap()
    OUT = nc.dram_tensor("OUT", [B, LO, D], F32, kind="ExternalOutput").ap()

    a2a_in = [nc.dram_tensor(f"a2a_in{b}", [N_CORES, 128, LO], BF16).ap()
              for b in range(B)]
    a2a_out = [nc.dram_tensor(f"a2a_out{b}", [N_CORES, 128, LO], BF16).ap()
               for b in range(B)]
    dn_bounce = [nc.dram_tensor(f"dn{b}", [2, 1, L], F32).ap()
                 for b in range(B)]

    with tile.TileContext(nc) as tc, ExitStack() as ctx:
        const = ctx.enter_context(tc.tile_pool(name="const", bufs=1))
        qkvt_pool = ctx.enter_context(tc.tile_pool(name="qkvt", bufs=2))
        xt_pool = ctx.enter_context(tc.tile_pool(name="xt", bufs=2))
        vp_pool = ctx.enter_context(tc.tile_pool(name="vp", bufs=2))
        p_pool = ctx.enter_context(tc.tile_pool(name="p", bufs=6))
        on_pool = ctx.enter_context(tc.tile_pool(name="on", bufs=2))
        rb_pool = ctx.enter_context(tc.tile_pool(name="rb", bufs=2))
        ot_pool = ctx.enter_context(tc.tile_pool(name="ot", bufs=2))
        otg_pool = ctx.enter_context(tc.tile_pool(name="otg", bufs=2))
        osb_pool = ctx.enter_context(tc.tile_pool(name="osb", bufs=2))

        ps_acc = ctx.enter_context(
            tc.tile_pool(name="ps_acc", bufs=1, space="PSUM"))
        ps_s = ctx.enter_context(
            tc.tile_pool(name="ps_s", bufs=2, space="PSUM"))
        ps_mm = ctx.enter_context(
            tc.tile_pool(name="ps_mm", bufs=2, space="PSUM"))

        # ---- constants / weights ----
        bq_sb = const.tile([128, 3], F32, tag="bq")
        for cc in range(3):
            nc.gpsimd.dma_start(out=bq_sb[:, cc:cc + 1], in_=BQKV[cc])
        wsb = const.tile([128, KT, 384], BF16, tag="wsb")
        for t in range(KT):
            nc.gpsimd.dma_start(out=wsb[:, t, :],
                                in_=WQKV[128 * t:128 * (t + 1), :])
        masks = const.tile([128, 128], BF16, tag="masks")
        nc.gpsimd.dma_start(out=masks, in_=MASKS)
        wout_sb = const.tile([128, KT, D], BF16, tag="wout")
        for t in range(KT):
            nc.gpsimd.dma_start(out=wout_sb[:, t, :],
                                in_=WOUT[128 * t:128 * (t + 1), :])
        bout_bc = const.tile([128, D], F32, tag="bout")
        nc.gpsimd.dma_start(
            out=bout_bc,
            in_=bass.AP(tensor=BOUT.tensor, offset=0, ap=[[0, 128], [1, D]]))

        def stage5(b):
            # ---- stage 5: output projection of batch b ----
            otg = otg_pool.tile([128, KT, LO], BF16, tag="otg")
            for si in range(N_CORES):
                nc.gpsimd.dma_start(out=otg[:, si, :], in_=a2a_out[b][si])
            for lt in range(LO // 128):
                for nk in range(D // 512):
                    po = ps_mm.tile([128, 512], F32, tag="mm")
                    for ct in range(KT):
                        nc.tensor.matmul(
                            po, otg[:, ct, 128 * lt:128 * (lt + 1)],
                            wout_sb[:, ct, 512 * nk:512 * (nk + 1)],
                            start=(ct == 0), stop=(ct == KT - 1))
                    osb = osb_pool.tile([128, 512], F32, tag="osb")
                    nc.vector.tensor_add(
                        osb, po, bout_bc[:, 512 * nk:512 * (nk + 1)])
                    nc.gpsimd.dma_start(
                        out=OUT[b, 128 * lt:128 * (lt + 1),
                                512 * nk:512 * (nk + 1)],
                        in_=osb)

        qkvts = {}
        vps = {}

        def stage12_gen(b):
            # ---- stage 1: X^T via xbar DMA + QKV projection (piecewise) ----
            # qkvt holds Q,K only; V goes to padded per-head staging tiles
            # (vsrc, [80, L]: rows 0:64 = V_h, row 64 = ones) so the dense
            # xbar block-transpose yields vph [128, 16, 80] whose
            # [:, jt, 0:65] slice is the contiguous [V_h | ones] PV
            # stationary (acc rows 0:64 = numerators, row 64 = denominator).
            # V_h1 is evicted at its natural partitions 64:128 (engines
            # cannot shift partitions) then DMA-shifted down to vsrc1[0:64].
            qkvt = qkvt_pool.tile([128, 2, L], BF16, tag="qkvt")
            qkvts[b] = qkvt
            vsrc0 = vp_pool.tile([80, L], BF16, tag="vsrc0")
            vsrc1 = vp_pool.tile([80, L], BF16, tag="vsrc1")
            vtmp = vp_pool.tile([128, L], BF16, tag="vtmp")
            nc.gpsimd.memset(vsrc0[64:65, :], 1.0)
            nc.gpsimd.memset(vsrc1[64:65, :], 1.0)
            for lc in range(L // 1024):
                xt = xt_pool.tile([128, KT, 1024], BF16, tag="xt")
                for t in range(KT):
                    nc.sync.dma_start_transpose(
                        out=xt[:, t, :],
                        in_=X[b, 1024 * lc:1024 * (lc + 1),
                              128 * t:128 * (t + 1)])
                for cc in range(3):
                    for nk in range(2):
                        c0 = 1024 * lc + 512 * nk
                        pq = ps_mm.tile([128, 512], F32, tag="mm")
                        for t in range(KT):
                            nc.tensor.matmul(
                                pq, wsb[:, t, 128 * cc:128 * (cc + 1)],
                                xt[:, t, 512 * nk:512 * (nk + 1)],
                                start=(t == 0), stop=(t == KT - 1))
                        if cc < 2:
                            nc.vector.tensor_scalar_add(
                                qkvt[:, cc, c0:c0 + 512],
                                pq, bq_sb[:, cc:cc + 1])
                        else:
                            nc.vector.tensor_scalar_add(
                                vsrc0[0:64, c0:c0 + 512],
                                pq[0:64, :], bq_sb[0:64, cc:cc + 1])
                            nc.vector.tensor_scalar_add(
                                vtmp[64:128, c0:c0 + 512],
                                pq[64:128, :], bq_sb[64:128, cc:cc + 1])
                            nc.gpsimd.dma_start(
                                out=vsrc1[0:64, c0:c0 + 512],
                                in_=vtmp[64:128, c0:c0 + 512])
                        yield
            # ---- stage 2: V' via per-head dense xbar block-transposes ----
            vph0 = vp_pool.tile([128, NJT, 80], BF16, tag="vph0")
            vph1 = vp_pool.tile([128, NJT, 80], BF16, tag="vph1")
            vps[b] = (vph0, vph1)
            nc.sync.dma_start_transpose(out=vph0, in_=vsrc0)
            nc.sync.dma_start_transpose(out=vph1, in_=vsrc1)
            yield

        for _ in stage12_gen(0):
            pass

        for b in range(B):
            qkvt = qkvts.pop(b)
            vph0, vph1 = vps.pop(b)
            # stage 1+2 of the next batch, interleaved into attention below
            nxt = stage12_gen(b + 1) if b + 1 < B else iter(())

            # ---- stage 3: attention per head ----
            for hp in range(2):
                hs = slice(64 * hp, 64 * (hp + 1))
                qh = qkvt[hs, 0, :]
                kh = qkvt[hs, 1, :]
                # PV stationary [V_h | ones] (see stage12_gen): numerators in
                # acc rows 0:64, denominator in row 64
                vph = vph0 if hp == 0 else vph1
                onum = on_pool.tile([65, L], F32, tag="onum")
                for q in range(NCH):
                    acc = ps_acc.tile([65, CQ], F32, tag="acc")
                    pending = None
                    last_jt = (CQ * (q + 1)) // 128 - 1
                    # last jt writing each 512-wide PSUM bank piece
                    last_for = [min(last_jt, NDT * q + bk * 4 + 3)
                                for bk in range(CQ // 512)]
                    for jt in range(last_jt + 1):
                        diag = jt // NDT == q
                        o = jt - NDT * q if diag else 0
                        c0 = 128 * o
                        # bank-aligned output pieces (<=512 f32 per bank)
                        pcs = []
                        lo = c0
                        while lo < CQ:
                            hi = min(CQ, (lo // 512 + 1) * 512)
                            pcs.append((lo, hi))
                            lo = hi
                        sp = ps_s.tile([128, CQ], F32, tag="s")
                        for (lo, hi) in pcs:
                            nc.tensor.matmul(
                                sp[:, lo:hi], kh[:, 128 * jt:128 * (jt + 1)],
                                qh[:, CQ * q + lo:CQ * q + hi],
                                start=True, stop=True)
                        psb = p_pool.tile([128, CQ], BF16, tag="p")
                        nc.scalar.activation(out=psb[:, c0:], in_=sp[:, c0:],
                                             func=ActF.Exp, scale=0.125)
                        if diag:
                            nc.vector.tensor_mul(
                                psb[:, c0:c0 + 128], psb[:, c0:c0 + 128],
                                masks)

                        def mm2(jt=jt, pcs=pcs, psb=psb):
                            for (lo, hi) in pcs:
                                bk = lo // 512
                                stop = jt == last_for[bk]
                                nc.tensor.matmul(
                                    acc[:, lo:hi], vph[:, jt, 0:65],
                                    psb[:, lo:hi],
                                    start=(jt == 0), stop=stop)
                                if stop:
                                    p0 = 512 * bk
                                    nc.vector.tensor_copy(
                                        onum[:, CQ * q + p0:CQ * q + p0 + 512],
                                        acc[0:65, p0:p0 + 512])
                        if pending is not None:
                            pending()
                        pending = mm2
                        if jt % 3 == 2:
                            next(nxt, None)
                    if pending is not None:
                        pending()
                    nc.gpsimd.dma_start(
                        out=dn_bounce[b][hp][:, CQ * q:CQ * (q + 1)],
                        in_=onum[64:65, CQ * q:CQ * (q + 1)])
                # broadcast raw denominators back, reciprocal, normalize
                rb = rb_pool.tile([64, L], F32, tag="rb")
                nc.gpsimd.dma_start(
                    out=rb,
                    in_=bass.AP(tensor=dn_bounce[b].tensor, offset=hp * L,
                                ap=[[0, 64], [1, L]]))
                nc.vector.reciprocal_approx_fast(out=rb, in_=rb)
                ot = ot_pool.tile([64, L], BF16, tag="ot")
                nc.vector.tensor_mul(ot, onum[0:64, :], rb)
                # write this head's slice of the A2A input as soon as ready
                for d in range(N_CORES):
                    nc.gpsimd.dma_start(
                        out=a2a_in[b][d, 64 * hp:64 * (hp + 1), :],
                        in_=ot[:, LO * d:LO * (d + 1)])
                if hp == 1:
                    nc.gpsimd.collective_compute(
                        "AllToAll", mybir.AluOpType.bypass,
                        replica_groups=[list(range(N_CORES))],
                        ins=[a2a_in[b]], outs=[a2a_out[b]])
                    if b > 0:
                        stage5(b - 1)
            for _ in nxt:
                pass

        stage5(B - 1)

    nc.compile()
    return nc


def make_inputs(X, W_qkv, b_qkv, W_out, b_out, B=BATCH, L=SEQ_LEN):
    """Build per-core input maps from full inputs."""
    Wr = np.ascontiguousarray(W_qkv).reshape(D_MODEL, N_HEADS, 3, D_ATTN)
    br = np.ascontiguousarray(b_qkv).reshape(N_HEADS, 3, D_ATTN)
    p = np.arange(128)[:, None]
    n = np.arange(128)[None, :]
    masks = (p <= n).astype(np.float32).astype(BF16_NP)
    Xb = np.ascontiguousarray(X, dtype=np.float32).astype(BF16_NP)
    Wo = np.ascontiguousarray(W_out, dtype=np.float32).astype(BF16_NP)
    in_maps = []
    for c in range(N_CORES):
        ha, hb = 2 * c, 2 * c + 1
        # column order per ctile: [Qa|Qb], [Ka|Kb], [Va|Vb]
        wp = np.concatenate(
            [Wr[:, ha, 0], Wr[:, hb, 0],
             Wr[:, ha, 1], Wr[:, hb, 1],
             Wr[:, ha, 2], Wr[:, hb, 2]], axis=1)
        bp = np.concatenate(
            [br[ha, 0], br[hb, 0], br[ha, 1], br[hb, 1],
             br[ha, 2], br[hb, 2]])
        in_maps.append({
            "X": Xb,
            "WQKV": np.ascontiguousarray(wp, dtype=np.float32)
                      .astype(BF16_NP),
            "BQKV": np.ascontiguousarray(bp, dtype=np.float32)
                      .reshape(3, 128, 1),
            "WOUT": Wo,
            "BOUT": np.ascontiguousarray(b_out, dtype=np.float32),
            "MASKS": masks,
        })
    return in_maps


def assemble_output(results, B=BATCH, L=SEQ_LEN):
    LO = L // N_CORES
    out = np.empty((B, L, D_MODEL), dtype=np.float32)
    for c in range(N_CORES):
        o = results[c]["OUT"]
        for b in range(B):
            out[b, LO * c:LO * (c + 1), :] = o[b]
    return out


_CACHED_NC = None


def kernel(X, W_qkv, b_qkv, W_out, b_out):
    global _CACHED_NC
    X = np.asarray(X, dtype=np.float32)
    if _CACHED_NC is None:
        _CACHED_NC = build_program(BATCH, SEQ_LEN)
    in_maps = make_inputs(X, np.asarray(W_qkv), np.asarray(b_qkv),
                          np.asarray(W_out), np.asarray(b_out))
    res = run_bass_kernel_spmd(_CACHED_NC, in_maps, list(range(N_CORES)))
    return assemble_output(res.results)


if __name__ == "__main__":
    nc = build_program(1, 2048)
    print("built + compiled ok")


# revision 23
# speedup vs baseline: 1.0720x; 1.0720x over previous
"""Causal multi-head attention on 8 TRN2 NeuronCores.

Sharding: tensor-parallel over heads (2 heads/core) for QKV projection and
attention; AllToAll redistributes attention outputs so each core owns L/8
rows of every batch for the output projection. Full inputs in, full output
out; all FLOPs on device.

v2 changes vs the PE-transpose baseline (456us):
  * X^T and V' are produced by the DMA xbar transpose engine
    (dma_start_transpose, HWDGE/sync queue) instead of PE transpose-mode
    matmuls. This removes 576 transposes + their LDWEIGHTS (~200us of PE
    time at the throttled clock) and stops transpose-mode activity from
    suppressing the HAM activity credit (transpose-mode never counts as
    PE-busy, so interleaving it everywhere pinned the PE clock at 1.2GHz).
  * ScalarE runs ONLY the softmax exp activations; PSUM evictions moved to
    DVE (tensor_copy), QKV bias-adds stay on DVE.
  * Softmax denominator reciprocal via reciprocal_approx_fast (~5x faster
    than InstReciprocal which measured 12.9us per [64, 2048] tile).
  * V' built per head by one dense SBUF->SBUF xbar block-transpose from a
    padded staging tile that carries a ones row, so [V_h | ones] is a
    contiguous PV stationary and the softmax denominator falls out of the
    PV matmul for free.

Per-core pipeline (per batch):
  1. X^T tiles via xbar transpose DMA straight from DRAM; QKV projection
     with stationary W tiles -> QKVt [384c, L], bias added on DVE.
  2. V' block-transpose (one xbar op) + ones columns.
  3. Causal attention per head: S^T = K-stationary.T @ Q into fp32 PSUM,
     exp fused on ScalarE (scale=1/sqrt(d), no max subtraction -- scores
     are bounded for this problem), diagonal tiles masked on DVE,
     O'.T accumulated over key tiles in fp32 PSUM with an appended ones
     row; normalization deferred: numerators evicted unnormalized (DVE),
     denominator reciprocal broadcast via DRAM bounce, one DVE multiply.
  4. AllToAll (per batch): own-128-channel x 256-row blocks redistributed
     so each core gets all 1024 channels for its own L/8 rows.
  5. Output projection (stationary gathered O.T tiles, moving W_out) plus
     broadcast fp32 bias, DMA to the per-core output row slice.
  The output projection of batch b-1 is emitted after batch b's AllToAll
  launch; stage 1+2 of batch b+1 interleaves into batch b's attention.
"""
from contextlib import ExitStack

import ml_dtypes
import numpy as np

import concourse.bass as bass
import concourse.tile as tile
from concourse import bacc, mybir
from concourse.bass_utils import run_bass_kernel_spmd

N_CORES = 8
D_MODEL = 1024
N_HEADS = 16
D_ATTN = 64
SEQ_LEN = 2048
BATCH = 4

F32 = mybir.dt.float32
BF16 = mybir.dt.bfloat16
ActF = mybir.ActivationFunctionType
BF16_NP = np.dtype(ml_dtypes.bfloat16)


def build_program(B=BATCH, L=SEQ_LEN):
    """Build the SPMD Bass program. Parametric over batch/seq for sim tests.
    Requires L % 1024 == 0. Full problem: B=4, L=2048."""
    assert L % 1024 == 0
    D = D_MODEL
    CQ = 1024                 # l_q chunk for attention
    NCH = L // CQ             # number of l_q chunks (2 at L=2048)
    NJT = L // 128            # number of l_k tiles (16)
    LO = L // N_CORES         # own rows per batch per core (256)
    KT = D // 128             # contraction k-tiles (8)
    NDT = CQ // 128           # l_k tiles per diagonal chunk (8)

    nc = bacc.Bacc("TRN2", target_bir_lowering=False, debug=False,
                   num_devices=N_CORES)

    X = nc.dram_tensor("X", [B, L, D], BF16, kind="ExternalInput").ap()
    WQKV = nc.dram_tensor("WQKV", [D, 384], BF16, kind="ExternalInput").ap()
    BQKV = nc.dram_tensor("BQKV", [3, 128, 1], F32, kind="ExternalInput").ap()
    WOUT = nc.dram_tensor("WOUT", [D, D], BF16, kind="ExternalInput").ap()
    BOUT = nc.dram_tensor("BOUT", [D], F32, kind="ExternalInput").ap()
    MASKS = nc.dram_tensor("MASKS", [128, 128], BF16,
                           kind="ExternalInput").ap()
    # V bounce buffers: kernel writes rows 0:64 (= V_h + bias), row 64 is
    # pre-staged to ones by the host, rows 65:80 are zero padding. Keeping
    # this in DRAM makes every xbar transpose DRAM->SBUF (no SBUF->SBUF
    # DMAs anywhere -- those force a deadlock-avoidance serialization
    # against DMA-transposes that convoys all DMA queues).
    VST = nc.dram_tensor("VST", [B, 2, 80, L], BF16,
                         kind="ExternalInput").ap()
    OUT = nc.dram_tensor("OUT", [B, LO, D], F32, kind="ExternalOutput").ap()

    a2a_in = [nc.dram_tensor(f"a2a_in{b}", [N_CORES, 128, LO], BF16).ap()
              for b in range(B)]
    a2a_out = [nc.dram_tensor(f"a2a_out{b}", [N_CORES, 128, LO], BF16).ap()
               for b in range(B)]
    dn_bounce = [nc.dram_tensor(f"dn{b}", [2, 1, L], F32).ap()
                 for b in range(B)]

    with tile.TileContext(nc) as tc, ExitStack() as ctx:
        const = ctx.enter_context(tc.tile_pool(name="const", bufs=1))
        qkvt_pool = ctx.enter_context(tc.tile_pool(name="qkvt", bufs=2))
        xt_pool = ctx.enter_context(tc.tile_pool(name="xt", bufs=2))
        vp_pool = ctx.enter_context(tc.tile_pool(name="vp", bufs=2))
        p_pool = ctx.enter_context(tc.tile_pool(name="p", bufs=6))
        on_pool = ctx.enter_context(tc.tile_pool(name="on", bufs=2))
        rb_pool = ctx.enter_context(tc.tile_pool(name="rb", bufs=2))
        ot_pool = ctx.enter_context(tc.tile_pool(name="ot", bufs=2))
        otg_pool = ctx.enter_context(tc.tile_pool(name="otg", bufs=2))
        osb_pool = ctx.enter_context(tc.tile_pool(name="osb", bufs=2))

        ps_acc = ctx.enter_context(
            tc.tile_pool(name="ps_acc", bufs=1, space="PSUM"))
        ps_s = ctx.enter_context(
            tc.tile_pool(name="ps_s", bufs=2, space="PSUM"))
        ps_mm = ctx.enter_context(
            tc.tile_pool(name="ps_mm", bufs=2, space="PSUM"))

        # ---- constants / weights ----
        bq_sb = const.tile([128, 3], F32, tag="bq")
        for cc in range(3):
            nc.gpsimd.dma_start(out=bq_sb[:, cc:cc + 1], in_=BQKV[cc])
        wsb = const.tile([128, KT, 384], BF16, tag="wsb")
        for t in range(KT):
            nc.gpsimd.dma_start(out=wsb[:, t, :],
                                in_=WQKV[128 * t:128 * (t + 1), :])
        masks = const.tile([128, 128], BF16, tag="masks")
        nc.gpsimd.dma_start(out=masks, in_=MASKS)
        wout_sb = const.tile([128, KT, D], BF16, tag="wout")
        for t in range(KT):
            nc.gpsimd.dma_start(out=wout_sb[:, t, :],
                                in_=WOUT[128 * t:128 * (t + 1), :])
        bout_bc = const.tile([128, D], F32, tag="bout")
        nc.gpsimd.dma_start(
            out=bout_bc,
            in_=bass.AP(tensor=BOUT.tensor, offset=0, ap=[[0, 128], [1, D]]))

        def stage5(b):
            # ---- stage 5: output projection of batch b ----
            otg = otg_pool.tile([128, KT, LO], BF16, tag="otg")
            for si in range(N_CORES):
                nc.gpsimd.dma_start(out=otg[:, si, :], in_=a2a_out[b][si])
            for lt in range(LO // 128):
                for nk in range(D // 512):
                    po = ps_mm.tile([128, 512], F32, tag="mm")
                    for ct in range(KT):
                        nc.tensor.matmul(
                            po, otg[:, ct, 128 * lt:128 * (lt + 1)],
                            wout_sb[:, ct, 512 * nk:512 * (nk + 1)],
                            start=(ct == 0), stop=(ct == KT - 1))
                    osb = osb_pool.tile([128, 512], F32, tag="osb")
                    nc.vector.tensor_add(
                        osb, po, bout_bc[:, 512 * nk:512 * (nk + 1)])
                    nc.gpsimd.dma_start(
                        out=OUT[b, 128 * lt:128 * (lt + 1),
                                512 * nk:512 * (nk + 1)],
                        in_=osb)

        qkvts = {}
        vps = {}

        def stage12_gen(b):
            # ---- stage 1: X^T via xbar DMA + QKV projection (piecewise) ----
            # qkvt holds Q,K only; V is staged in vstage (its natural
            # partition layout), bounced to the DRAM VST buffers whose row 64
            # is pre-staged ones, then block-transposed DRAM->SBUF into
            # vph [128, 16, 80]: the [:, jt, 0:65] slice is the contiguous
            # [V_h | ones] PV stationary (acc rows 0:64 = numerators,
            # row 64 = denominator).
            qkvt = qkvt_pool.tile([128, 2, L], BF16, tag="qkvt")
            qkvts[b] = qkvt
            vstage = vp_pool.tile([128, L], BF16, tag="vstage")
            for lc in range(L // 1024):
                xt = xt_pool.tile([128, KT, 1024], BF16, tag="xt")
                for t in range(KT):
                    nc.sync.dma_start_transpose(
                        out=xt[:, t, :],
                        in_=X[b, 1024 * lc:1024 * (lc + 1),
                              128 * t:128 * (t + 1)])
                for cc in range(3):
                    for nk in range(2):
                        c0 = 1024 * lc + 512 * nk
                        pq = ps_mm.tile([128, 512], F32, tag="mm")
                        for t in range(KT):
                            nc.tensor.matmul(
                                pq, wsb[:, t, 128 * cc:128 * (cc + 1)],
                                xt[:, t, 512 * nk:512 * (nk + 1)],
                                start=(t == 0), stop=(t == KT - 1))
                        if cc < 2:
                            nc.vector.tensor_scalar_add(
                                qkvt[:, cc, c0:c0 + 512],
                                pq, bq_sb[:, cc:cc + 1])
                        else:
                            nc.vector.tensor_scalar_add(
                                vstage[:, c0:c0 + 512],
                                pq, bq_sb[:, cc:cc + 1])
                            nc.gpsimd.dma_start(
                                out=VST[b, 0, 0:64, c0:c0 + 512],
                                in_=vstage[0:64, c0:c0 + 512])
                            nc.gpsimd.dma_start(
                                out=VST[b, 1, 0:64, c0:c0 + 512],
                                in_=vstage[64:128, c0:c0 + 512])
                        yield
            # ---- stage 2: V' via per-head dense xbar block-transposes ----
            vph0 = vp_pool.tile([128, NJT, 80], BF16, tag="vph0")
            vph1 = vp_pool.tile([128, NJT, 80], BF16, tag="vph1")
            vps[b] = (vph0, vph1)
            nc.sync.dma_start_transpose(out=vph0, in_=VST[b, 0])
            nc.sync.dma_start_transpose(out=vph1, in_=VST[b, 1])
            yield

        for _ in stage12_gen(0):
            pass

        for b in range(B):
            qkvt = qkvts.pop(b)
            vph0, vph1 = vps.pop(b)
            # stage 1+2 of the next batch, interleaved into attention below
            nxt = stage12_gen(b + 1) if b + 1 < B else iter(())

            # ---- stage 3: attention per head ----
            for hp in range(2):
                hs = slice(64 * hp, 64 * (hp + 1))
                qh = qkvt[hs, 0, :]
                kh = qkvt[hs, 1, :]
                # PV stationary [V_h | ones] (see stage12_gen): numerators in
                # acc rows 0:64, denominator in row 64
                vph = vph0 if hp == 0 else vph1
                onum = on_pool.tile([65, L], F32, tag="onum")
                for q in range(NCH):
                    acc = ps_acc.tile([65, CQ], F32, tag="acc")
                    pending = None
                    last_jt = (CQ * (q + 1)) // 128 - 1
                    # last jt writing each 512-wide PSUM bank piece
                    last_for = [min(last_jt, NDT * q + bk * 4 + 3)
                                for bk in range(CQ // 512)]
                    for jt in range(last_jt + 1):
                        diag = jt // NDT == q
                        o = jt - NDT * q if diag else 0
                        c0 = 128 * o
                        # bank-aligned output pieces (<=512 f32 per bank)
                        pcs = []
                        lo = c0
                        while lo < CQ:
                            hi = min(CQ, (lo // 512 + 1) * 512)
                            pcs.append((lo, hi))
                            lo = hi
                        sp = ps_s.tile([128, CQ], F32, tag="s")
                        for (lo, hi) in pcs:
                            nc.tensor.matmul(
                                sp[:, lo:hi], kh[:, 128 * jt:128 * (jt + 1)],
                                qh[:, CQ * q + lo:CQ * q + hi],
                                start=True, stop=True)
                        psb = p_pool.tile([128, CQ], BF16, tag="p")
                        nc.scalar.activation(out=psb[:, c0:], in_=sp[:, c0:],
                                             func=ActF.Exp, scale=0.125)
                        if diag:
                            nc.vector.tensor_mul(
                                psb[:, c0:c0 + 128], psb[:, c0:c0 + 128],
                                masks)

                        def mm2(jt=jt, pcs=pcs, psb=psb):
                            for (lo, hi) in pcs:
                                bk = lo // 512
                                stop = jt == last_for[bk]
                                nc.tensor.matmul(
                                    acc[:, lo:hi], vph[:, jt, 0:65],
                                    psb[:, lo:hi],
                                    start=(jt == 0), stop=stop)
                                if stop:
                                    p0 = 512 * bk
                                    nc.vector.tensor_copy(
                                        onum[:, CQ * q + p0:CQ * q + p0 + 512],
                                        acc[0:65, p0:p0 + 512])
                        if pending is not None:
                            pending()
                        pending = mm2
                        if jt % 3 == 2:
                            next(nxt, None)
                    if pending is not None:
                        pending()
                    nc.gpsimd.dma_start(
                        out=dn_bounce[b][hp][:, CQ * q:CQ * (q + 1)],
                        in_=onum[64:65, CQ * q:CQ * (q + 1)])
                # broadcast raw denominators back, reciprocal, normalize
                rb = rb_pool.tile([64, L], F32, tag="rb")
                nc.gpsimd.dma_start(
                    out=rb,
                    in_=bass.AP(tensor=dn_bounce[b].tensor, offset=hp * L,
                                ap=[[0, 64], [1, L]]))
                nc.vector.reciprocal_approx_fast(out=rb, in_=rb)
                ot = ot_pool.tile([64, L], BF16, tag="ot")
                nc.vector.tensor_mul(ot, onum[0:64, :], rb)
                # write this head's slice of the A2A input as soon as ready
                for d in range(N_CORES):
                    nc.gpsimd.dma_start(
                        out=a2a_in[b][d, 64 * hp:64 * (hp + 1), :],
                        in_=ot[:, LO * d:LO * (d + 1)])
                if hp == 1:
                    nc.gpsimd.collective_compute(
                        "AllToAll", mybir.AluOpType.bypass,
                        replica_groups=[list(range(N_CORES))],
                        ins=[a2a_in[b]], outs=[a2a_out[b]])
                    if b > 0:
                        stage5(b - 1)
            for _ in nxt:
                pass

        stage5(B - 1)

    nc.compile()
    return nc


def make_inputs(X, W_qkv, b_qkv, W_out, b_out, B=BATCH, L=SEQ_LEN):
    """Build per-core input maps from full inputs."""
    Wr = np.ascontiguousarray(W_qkv).reshape(D_MODEL, N_HEADS, 3, D_ATTN)
    br = np.ascontiguousarray(b_qkv).reshape(N_HEADS, 3, D_ATTN)
    vst = np.zeros((B, 2, 80, L), dtype=np.float32)
    vst[:, :, 64, :] = 1.0
    vst = vst.astype(BF16_NP)
    p = np.arange(128)[:, None]
    n = np.arange(128)[None, :]
    masks = (p <= n).astype(np.float32).astype(BF16_NP)
    Xb = np.ascontiguousarray(X, dtype=np.float32).astype(BF16_NP)
    Wo = np.ascontiguousarray(W_out, dtype=np.float32).astype(BF16_NP)
    in_maps = []
    for c in range(N_CORES):
        ha, hb = 2 * c, 2 * c + 1
        # column order per ctile: [Qa|Qb], [Ka|Kb], [Va|Vb]
        wp = np.concatenate(
            [Wr[:, ha, 0], Wr[:, hb, 0],
             Wr[:, ha, 1], Wr[:, hb, 1],
             Wr[:, ha, 2], Wr[:, hb, 2]], axis=1)
        bp = np.concatenate(
            [br[ha, 0], br[hb, 0], br[ha, 1], br[hb, 1],
             br[ha, 2], br[hb, 2]])
        in_maps.append({
            "X": Xb,
            "WQKV": np.ascontiguousarray(wp, dtype=np.float32)
                      .astype(BF16_NP),
            "BQKV": np.ascontiguousarray(bp, dtype=np.float32)
                      .reshape(3, 128, 1),
            "WOUT": Wo,
            "BOUT": np.ascontiguousarray(b_out, dtype=np.float32),
            "MASKS": masks,
            "VST": vst,
        })
    return in_maps


def assemble_output(results, B=BATCH, L=SEQ_LEN):
    LO = L // N_CORES
    out = np.empty((B, L, D_MODEL), dtype=np.float32)
    for c in range(N_CORES):
        o = results[c]["OUT"]
        for b in range(B):
            out[b, LO * c:LO * (c + 1), :] = o[b]
    return out


_CACHED_NC = None


def kernel(X, W_qkv, b_qkv, W_out, b_out):
    global _CACHED_NC
    X = np.asarray(X, dtype=np.float32)
    if _CACHED_NC is None:
        _CACHED_NC = build_program(BATCH, SEQ_LEN)
    in_maps = make_inputs(X, np.asarray(W_qkv), np.asarray(b_qkv),
                          np.asarray(W_out), np.asarray(b_out))
    res = run_bass_kernel_spmd(_CACHED_NC, in_maps, list(range(N_CORES)))
    return assemble_output(res.results)


if __name__ == "__main__":
    nc = build_program(1, 2048)
    print("built + compiled ok")
